# revision 1
# baseline (speedup 1.0000x reference)
"""Trainium2 Bass kernel for nn_DeformationCorrector.

Math (per particle, F = [[a,b],[c,d]], det F > 0 for this data):
  closed-form 2x2 SVD:  y1 = (a+d)^2 + (c-b)^2,  y2 = (a-d)^2 + (c+b)^2
    sq1 = sqrt(y1), sq2 = sqrt(y2);  sigma1 = (sq1+sq2)/2, sigma2 = (sq1-sq2)/2
  polar rotation R = U@Vh = [[p,-q],[q,p]],  p = (a+d)/sq1, q = (c-b)/sq1
  features (dedup; -1 shifts folded into b1):
    [sq1+sq2, sq1-sq2, a^2+c^2, ab+cd, b^2+d^2, ad-bc]  @ W1eff + b1eff
  MLP 6->128->128->3 (symmetrized W3), then delta = R @ x_sym, out = delta + F.

Distribution: pure data parallel over 8 cores, contiguous shards, weights
replicated. Layout conversions (particle-major elementwise <-> feature-major
matmul) go through cheap DRAM round trips instead of on-chip transposes.
"""

import os
from contextlib import ExitStack

import numpy as np

import concourse.bass as bass
import concourse.bacc as bacc
import concourse.tile as tile
from concourse.tile_rust import add_dep_helper
from concourse import mybir
from concourse.bass_utils import run_bass_kernel_spmd

NCORES = 8
P = 128
T = 512                 # matmul moving free dim (one PSUM bank of fp32)
CB = 512                # particles per partition per block
BLK = P * CB            # 65536 particles per block
NBLK = 2
NPC = NBLK * BLK        # 131072 particles per core (padded)
NTOT = NCORES * NPC     # 1048576
N = 1_000_000
HID = 128

CHUNKS_PER_BLK = BLK // T      # 128
GROUPS_PER_BLK = CHUNKS_PER_BLK // 4   # 32 (4 chunks per group: row/col packing)

FP32 = mybir.dt.float32
F32R = mybir.dt.float32r
BF16 = mybir.dt.bfloat16
AF = mybir.ActivationFunctionType
OP = mybir.AluOpType

# fraction of relu chunks handled by ACT (rest on DVE); x-drain alternates
RELU_ACT_OF10 = int(os.environ.get("K_RELU_ACT_OF10", "5"))

_built = {}
_last_results = None


def _relu_on_act(c):
    return (c % 10) < RELU_ACT_OF10


def build_program(nblk=NBLK, cb=CB, dbg=False):
    # local size overrides (for simulation/testing)
    global NBLK, CB, BLK, NPC, CHUNKS_PER_BLK, GROUPS_PER_BLK
    NBLK_s, CB_s = NBLK, CB
    NBLK, CB = nblk, cb
    BLK_l = P * CB
    NPC_l = NBLK * BLK_l
    try:
        nc = _build_impl(NBLK, CB, BLK_l, NPC_l, dbg)
    finally:
        NBLK, CB = NBLK_s, CB_s
    return nc


def _build_impl(NBLK, CB, BLK, NPC, dbg=False):
    assert CB == T, 'g-major DRAM layout requires CB == T'
    CHUNKS_PER_BLK = BLK // T
    GROUPS_PER_BLK = CHUNKS_PER_BLK // 4
    nc = bacc.Bacc(trn_type="TRN2")

    F_in = nc.dram_tensor("F", [NPC, 4], FP32, kind="ExternalInput")
    W1S_in = nc.dram_tensor("W1S", [P, P], BF16, kind="ExternalInput")
    W2_in = nc.dram_tensor("W2", [P, P], BF16, kind="ExternalInput")
    W3S_in = nc.dram_tensor("W3S", [P, 32], BF16, kind="ExternalInput")
    B1_in = nc.dram_tensor("B1", [P, 1], FP32, kind="ExternalInput")
    B2_in = nc.dram_tensor("B2", [P, 1], FP32, kind="ExternalInput")
    B3S_in = nc.dram_tensor("B3S", [P, 1], FP32, kind="ExternalInput")
    OUT = nc.dram_tensor("OUT", [NPC, 4], FP32, kind="ExternalOutput")
    if dbg:
        FEATD = nc.dram_tensor("FEATD", [24, BLK // 4], FP32, kind="ExternalOutput")
        XD = nc.dram_tensor("XD", [12, BLK // 4], FP32, kind="ExternalOutput")
        PQ = nc.dram_tensor("PQ", [2 * P, CB], FP32, kind="ExternalOutput")
        H1D = nc.dram_tensor("H1D", [P, T], FP32, kind="ExternalOutput")

    with tile.TileContext(nc) as tc, ExitStack() as ctx:
        consts = ctx.enter_context(tc.tile_pool(name="consts", bufs=1))
        fblk = ctx.enter_context(tc.tile_pool(name="fblk", bufs=NBLK))
        scr = ctx.enter_context(tc.tile_pool(name="scr", bufs=1))
        featp = ctx.enter_context(tc.tile_pool(name="featp", bufs=NBLK))
        dramp = ctx.enter_context(tc.tile_pool(name="dramp", bufs=NBLK, space="DRAM"))
        fmp = ctx.enter_context(tc.tile_pool(name="fmp", bufs=2))
        hp = ctx.enter_context(tc.tile_pool(name="hp", bufs=6))
        xp = ctx.enter_context(tc.tile_pool(name="xp", bufs=2))
        outp = ctx.enter_context(tc.tile_pool(name="outp", bufs=2))
        psz1 = ctx.enter_context(tc.tile_pool(name="psz1", bufs=4, space="PSUM"))
        psz2 = ctx.enter_context(tc.tile_pool(name="psz2", bufs=3, space="PSUM"))
        psx = ctx.enter_context(tc.tile_pool(name="psx", bufs=1, space="PSUM"))

        # ---- constants ----
        w1s_sb = consts.tile([P, P], BF16)
        nc.sync.dma_start(out=w1s_sb[:], in_=W1S_in[:, :])
        w2_sb = consts.tile([P, P], BF16)
        nc.sync.dma_start(out=w2_sb[:], in_=W2_in[:, :])
        w3s_sb = consts.tile([P, 32], BF16)
        nc.sync.dma_start(out=w3s_sb[:], in_=W3S_in[:, :])
        b1_sb = consts.tile([P, 1], FP32)
        nc.sync.dma_start(out=b1_sb[:], in_=B1_in[:, :])
        b2_sb = consts.tile([P, 1], FP32)
        nc.sync.dma_start(out=b2_sb[:], in_=B2_in[:, :])
        b3s_sb = consts.tile([P, 1], FP32)
        nc.sync.dma_start(out=b3s_sb[:], in_=B3S_in[:, :])

        h1_dbg = []
        f_tiles = []
        p_tiles = []
        q_tiles = []
        featd_tiles = []
        xd_tiles = []

        # ============ stage 1: particle-major features (in column halves) ============
        H = CB // 2
        for b in range(NBLK):
            f_sb = fblk.tile([P, 4 * CB], FP32, tag="F", name=f"f_sb{b}")
            F_bv = F_in[:, :].rearrange("(b i g j) k -> b i g (j k)", b=NBLK, i=32, g=4)[b]
            for g in range(4):
                nc.sync.dma_start(out=f_sb[32 * g : 32 * g + 32, :], in_=F_bv[:, g, :])
            f_tiles.append(f_sb)
            fr = f_sb.rearrange("p (c k) -> p c k", k=4)
            fr2 = f_sb.rearrange("p (c k2 k) -> p c k2 k", k2=2, k=2)

            feat_sb = featp.tile([P, 6 * CB], FP32, tag="feat", name=f"feat_sb{b}")
            fv = feat_sb.rearrange("p (f c) -> p f c", f=6)
            sq_sb = scr.tile([P, 4 * CB], FP32, tag="sq", name=f"sq_sb{b}")
            sqr = sq_sb.rearrange("p (c k) -> p c k", k=4)
            pp_sb = scr.tile([P, 2 * CB], FP32, tag="pp", name=f"pp_sb{b}")
            ppv = pp_sb.rearrange("p (c k2) -> p c k2", k2=2)
            ad_sb = scr.tile([P, CB], FP32, tag="ad", name=f"ad_sb{b}")
            bc_sb = scr.tile([P, CB], FP32, tag="bc", name=f"bc_sb{b}")
            m_sb = scr.tile([P, CB], FP32, tag="m", name=f"m_sb{b}")
            y1_sb = scr.tile([P, CB], FP32, tag="y1", name=f"y1_sb{b}")
            y2_sb = scr.tile([P, CB], FP32, tag="y2", name=f"y2_sb{b}")
            sq1_sb = scr.tile([P, CB], FP32, tag="sq1", name=f"sq1_sb{b}")
            sq2_sb = scr.tile([P, CB], FP32, tag="sq2", name=f"sq2_sb{b}")
            s_sb = scr.tile([P, CB], FP32, tag="s", name=f"s_sb{b}")
            v_sb = scr.tile([P, CB], FP32, tag="v", name=f"v_sb{b}")
            rinv_sb = scr.tile([P, CB], FP32, tag="rinv", name=f"rinv_sb{b}")
            p_sb = fblk.tile([P, CB], FP32, tag="p", name=f"p_sb{b}")
            q_sb = fblk.tile([P, CB], FP32, tag="q", name=f"q_sb{b}")
            p_tiles.append(p_sb)
            q_tiles.append(q_sb)

            featd = dramp.tile([24, BLK // 4], BF16, tag="featd", name=f"featd{b}")
            featd_tiles.append(featd)

            for h in range(2):
                cs = slice(h * H, (h + 1) * H)
                av, bv_, cv, dv = (fr[:, cs, k] for k in range(4))
                ac = fr2[:, cs, :, 0]
                bd = fr2[:, cs, :, 1]
                aa, bb, cc, dd = (sqr[:, cs, k] for k in range(4))
                # squares on gpsimd (strided views of the half)
                nc.gpsimd.tensor_tensor(
                    out=sq_sb.rearrange("p (c k) -> p c k", k=4)[:, cs, :],
                    in0=fr[:, cs, :], in1=fr[:, cs, :], op=OP.mult)
                nc.vector.tensor_tensor(out=ppv[:, cs, :], in0=ac, in1=bd, op=OP.mult)
                nc.vector.tensor_tensor(out=fv[:, 3, cs], in0=ppv[:, cs, 0], in1=ppv[:, cs, 1], op=OP.add)
                nc.vector.tensor_tensor(out=ad_sb[:, cs], in0=av, in1=dv, op=OP.mult)
                nc.gpsimd.tensor_tensor(out=bc_sb[:, cs], in0=bv_, in1=cv, op=OP.mult)
                nc.vector.tensor_tensor(out=fv[:, 5, cs], in0=ad_sb[:, cs], in1=bc_sb[:, cs], op=OP.subtract)
                nc.vector.tensor_tensor(out=fv[:, 2, cs], in0=aa, in1=cc, op=OP.add)
                nc.vector.tensor_tensor(out=fv[:, 4, cs], in0=bb, in1=dd, op=OP.add)
                nc.vector.tensor_tensor(out=m_sb[:, cs], in0=fv[:, 2, cs], in1=fv[:, 4, cs], op=OP.add)
                nc.vector.scalar_tensor_tensor(
                    out=y1_sb[:, cs], in0=fv[:, 5, cs], scalar=2.0, in1=m_sb[:, cs],
                    op0=OP.mult, op1=OP.add)
                nc.vector.scalar_tensor_tensor(
                    out=y2_sb[:, cs], in0=fv[:, 5, cs], scalar=-2.0, in1=m_sb[:, cs],
                    op0=OP.mult, op1=OP.add)
                nc.vector.tensor_scalar(
                    out=y2_sb[:, cs], in0=y2_sb[:, cs], scalar1=0.0, scalar2=None, op0=OP.max)
                nc.scalar.activation(out=sq1_sb[:, cs], in_=y1_sb[:, cs], func=AF.Sqrt)
                nc.scalar.activation(out=sq2_sb[:, cs], in_=y2_sb[:, cs], func=AF.Sqrt)
                nc.vector.tensor_tensor(out=fv[:, 0, cs], in0=sq1_sb[:, cs], in1=sq2_sb[:, cs], op=OP.add)
                nc.vector.tensor_tensor(out=fv[:, 1, cs], in0=sq1_sb[:, cs], in1=sq2_sb[:, cs], op=OP.subtract)
                nc.vector.tensor_tensor(out=s_sb[:, cs], in0=av, in1=dv, op=OP.add)
                nc.vector.tensor_tensor(out=v_sb[:, cs], in0=cv, in1=bv_, op=OP.subtract)
                nc.vector.reciprocal_approx_fast(out=rinv_sb[:, cs], in_=sq1_sb[:, cs])
                nc.vector.tensor_tensor(out=p_sb[:, cs], in0=s_sb[:, cs], in1=rinv_sb[:, cs], op=OP.mult)
                nc.vector.tensor_tensor(out=q_sb[:, cs], in0=v_sb[:, cs], in1=rinv_sb[:, cs], op=OP.mult)
                for g in range(4):
                    nc.gpsimd.dma_start(
                        out=featd[6 * g : 6 * g + 6, :].rearrange("f (i j) -> i f j", j=T)[:, :, cs],
                        in_=feat_sb[32 * g : 32 * g + 32, :].rearrange("i (f j) -> i f j", j=T)[:, :, cs],
                    )

            xd = dramp.tile([12, BLK // 4], FP32, tag="xd", name=f"xd{b}")
            xd_tiles.append(xd)


        # ============ stage 2: feature-major MLP ============
        # 4-group superblocks (8192 particles): one fancy DMA in, one out.
        # WAR safety for the fancy (untracked) APs is enforced with explicit
        # dep edges against the previous user of the same buffer slot.
        SB = 4  # groups per superblock
        cglobal = 0
        n_super = GROUPS_PER_BLK // SB
        for s_outer in range(n_super * NBLK):
            b = s_outer % NBLK
            s = s_outer // NBLK
            featd = featd_tiles[b]
            xd = xd_tiles[b]
            if True:
                # ---- featfm load: [4 groups][6 feats] -> partitions 32g+f ----
                featfm = fmp.tile([P, SB * T], BF16, tag="featfm", name=f"ffm{b}_{s}")
                ffm_gv = featfm.rearrange("(g r) c -> g r c", g=4)
                for g in range(4):
                    nc.sync.dma_start(
                        out=featfm[32 * g : 32 * g + 6, :],
                        in_=featd[6 * g : 6 * g + 6, SB * T * s : SB * T * (s + 1)],
                    )

                x_sb = xp.tile([P, SB * T], FP32, tag="xsb", name=f"xsb{b}_{s}")
                for i2 in range(SB):
                    i = s * SB + i2
                    # ---- L1: 4 row-tiled matmuls, one PSUM bank each (4-way overlap) ----
                    z1s = [
                        psz1.tile([P, T], FP32, tag="z1", name=f"z1_{b}_{i}_{g}")
                        for g in range(4)
                    ]
                    for g in range(4):
                        nc.tensor.matmul(
                            out=z1s[g][:],
                            lhsT=w1s_sb[32 * g : 32 * g + 6, :],
                            rhs=ffm_gv[g, :6, i2 * T : (i2 + 1) * T],
                            tile_position=(32 * g, 0),
                        )
                    h1p = [
                        hp.tile([P, 2 * T], BF16, tag="h1", name=f"h1_{b}_{i}_0"),
                        hp.tile([P, 2 * T], BF16, tag="h1", name=f"h1_{b}_{i}_1"),
                    ]
                    for g in range(4):
                        hview = h1p[g // 2][:, (g % 2) * T : (g % 2 + 1) * T]
                        if (cglobal + g) % 2 == 0:
                            nc.scalar.activation(
                                out=hview, in_=z1s[g][:], func=AF.Relu, bias=b1_sb[:]
                            )
                        else:
                            nc.vector.tensor_scalar(
                                out=hview, in0=z1s[g][:], scalar1=b1_sb[:],
                                scalar2=0.0, op0=OP.add, op1=OP.max,
                            )
                    if dbg and b == 0 and i == 0:
                        h1_dbg.append(h1p[0])
                    # ---- L2 + relu2 + L3 per chunk ----
                    x_ps = psx.tile([P, T], FP32, tag="x", name=f"xps{b}_{i}")
                    for g in range(4):
                        z2 = psz2.tile([P, T], FP32, tag="z2", name=f"z2_{b}_{i}_{g}")
                        nc.tensor.matmul(
                            out=z2[:], lhsT=w2_sb[:],
                            rhs=h1p[g // 2][:, (g % 2) * T : (g % 2 + 1) * T],
                        )
                        h2 = hp.tile([P, T], BF16, tag="h2", name=f"h2_{b}_{i}_{g}", bufs=8)
                        if (cglobal + g) % 2 == 0:
                            nc.scalar.activation(
                                out=h2[:], in_=z2[:], func=AF.Relu, bias=b2_sb[:]
                            )
                        else:
                            nc.vector.tensor_scalar(
                                out=h2[:], in0=z2[:], scalar1=b2_sb[:],
                                scalar2=0.0, op0=OP.add, op1=OP.max,
                            )
                        nc.tensor.matmul(
                            out=x_ps[32 * g : 32 * g + 32, :],
                            lhsT=w3s_sb[:, :],
                            rhs=h2[:],
                            tile_position=(0, 32 * g),
                        )
                    # ---- x drain into superblock x_sb ----
                    dr = nc.scalar.activation(
                        out=x_sb[:, i2 * T : (i2 + 1) * T], in_=x_ps[:],
                        func=AF.Identity, bias=b3s_sb[:],
                    )
                    cglobal += 1
                # ---- x -> DRAM: one fancy DMA per superblock ----
                for g in range(4):
                    nc.scalar.dma_start(
                        out=xd[3 * g : 3 * g + 3, SB * T * s : SB * T * (s + 1)],
                        in_=x_sb[32 * g : 32 * g + 3, :],
                    )

        if dbg:
            nc.gpsimd.dma_start(out=FEATD[:, :], in_=featd_tiles[0][:, :])
            nc.sync.dma_start(out=XD[:, :], in_=xd_tiles[0][:, :])
            nc.sync.dma_start(out=PQ[:P, :], in_=p_tiles[0][:])
            nc.sync.dma_start(out=PQ[P:, :], in_=q_tiles[0][:])
            nc.gpsimd.dma_start(out=H1D[:, :], in_=h1_dbg[0][:, :T])

        # ============ stage 3: particle-major backend ============
        for b in range(NBLK):
            xd = xd_tiles[b]
            f_sb = f_tiles[b]
            p_sb = p_tiles[b]
            q_sb = q_tiles[b]
            fr = f_sb.rearrange("p (c k) -> p c k", k=4)

            xs_all = xp.tile([P, 3 * CB], FP32, tag="xsall", name=f"xsall{b}")
            xs_v = xs_all.rearrange("p (k c) -> p k c", k=3)
            for g in range(4):
                nc.sync.dma_start(
                    out=xs_all[32 * g : 32 * g + 32, :].rearrange("i (k j) -> i k j", j=T),
                    in_=xd[3 * g : 3 * g + 3, :].rearrange("k (i j) -> i k j", j=T),
                )
            # Pall = p * [x0 x1 x2] on gpsimd ; Qall on vector (tail parallelism)
            pall = scr.tile([P, 3 * CB], FP32, tag="pall", name=f"pall{b}")
            pall_v = pall.rearrange("p (k c) -> p k c", k=3)
            nc.gpsimd.tensor_tensor(
                out=pall[:], in0=xs_all[:],
                in1=p_sb[:].unsqueeze(1).to_broadcast([P, 3, CB]), op=OP.mult,
            )
            qall = scr.tile([P, 3 * CB], FP32, tag="qall", name=f"qall{b}")
            qall_v = qall.rearrange("p (k c) -> p k c", k=3)
            nc.vector.tensor_tensor(
                out=qall[:], in0=xs_all[:],
                in1=q_sb[:].unsqueeze(1).to_broadcast([P, 3, CB]), op=OP.mult,
            )
            out_sb = outp.tile([P, 4 * CB], FP32, tag="out", name=f"out_sb{b}")
            ov = out_sb.rearrange("p (c k) -> p c k", k=4)
            t0 = scr.tile([P, CB], FP32, tag="t0", name=f"t0_{b}")
            nc.gpsimd.tensor_tensor(out=t0[:], in0=pall_v[:, 0], in1=qall_v[:, 1], op=OP.subtract)
            nc.gpsimd.tensor_tensor(out=ov[:, :, 0], in0=t0[:], in1=fr[:, :, 0], op=OP.add)
            t1 = scr.tile([P, CB], FP32, tag="t1", name=f"t1_{b}")
            nc.vector.tensor_tensor(out=t1[:], in0=pall_v[:, 1], in1=qall_v[:, 2], op=OP.subtract)
            nc.vector.tensor_tensor(out=ov[:, :, 1], in0=t1[:], in1=fr[:, :, 1], op=OP.add)
            t2 = scr.tile([P, CB], FP32, tag="t2", name=f"t2_{b}")
            nc.gpsimd.tensor_tensor(out=t2[:], in0=qall_v[:, 0], in1=pall_v[:, 1], op=OP.add)
            nc.gpsimd.tensor_tensor(out=ov[:, :, 2], in0=t2[:], in1=fr[:, :, 2], op=OP.add)
            t3 = scr.tile([P, CB], FP32, tag="t3", name=f"t3_{b}")
            nc.vector.tensor_tensor(out=t3[:], in0=qall_v[:, 1], in1=pall_v[:, 2], op=OP.add)
            nc.vector.tensor_tensor(out=ov[:, :, 3], in0=t3[:], in1=fr[:, :, 3], op=OP.add)

            OUT_bv = OUT[:, :].rearrange("(b i g j) k -> b i g (j k)", b=NBLK, i=32, g=4)[b]
            for g in range(4):
                nc.sync.dma_start(out=OUT_bv[:, g, :], in_=out_sb[32 * g : 32 * g + 32, :])

    nc.finalize()
    return nc


def prep_weights(W1, b1, W2, b2, W3, b3):
    """Host-side weight transforms (tiny)."""
    W1 = np.asarray(W1, np.float32)
    b1 = np.asarray(b1, np.float32)
    W2 = np.asarray(W2, np.float32)
    b2 = np.asarray(b2, np.float32)
    W3 = np.asarray(W3, np.float32)
    b3 = np.asarray(b3, np.float32)
    # features: [sq1+sq2, sq1-sq2, f2, f3, f4, f5]
    W1eff = np.stack(
        [0.5 * W1[0], 0.5 * W1[1], W1[2], W1[3] + W1[4], W1[5], W1[6]], axis=0
    )  # [6, 128]
    b1eff = b1 - (W1[0] + W1[1] + W1[2] + W1[5] + W1[6])
    W1S = np.zeros((P, P), np.float32)
    for g in range(4):
        W1S[32 * g : 32 * g + 6, :] = W1eff
    # symmetrized third layer: x_sym = [x00, (x01+x10)/2, x11]
    W3S = np.zeros((P, 32), np.float32)
    W3S[:, 0] = W3[:, 0]
    W3S[:, 1] = 0.5 * (W3[:, 1] + W3[:, 2])
    W3S[:, 2] = W3[:, 3]
    b3S3 = np.array([b3[0], 0.5 * (b3[1] + b3[2]), b3[3]], np.float32)
    B3S = np.zeros((P, 1), np.float32)
    for j in range(4):
        B3S[32 * j : 32 * j + 3, 0] = b3S3
    import ml_dtypes
    return {
        "W1S": W1S.astype(ml_dtypes.bfloat16),
        "W2": W2.astype(ml_dtypes.bfloat16),
        "W3S": W3S.astype(ml_dtypes.bfloat16),
        "B1": b1eff.reshape(P, 1).astype(np.float32),
        "B2": b2.reshape(P, 1).astype(np.float32),
        "B3S": B3S,
    }


def kernel(F, W1, b1, W2, b2, W3, b3):
    global _last_results
    F = np.asarray(F, np.float32).reshape(-1, 4)
    n = F.shape[0]
    assert n == N, f"expected {N} particles, got {n}"

    if "nc" not in _built:
        _built["nc"] = build_program()
    nc = _built["nc"]

    wmaps = prep_weights(W1, b1, W2, b2, W3, b3)
    Fpad = np.empty((NTOT, 4), np.float32)
    Fpad[:n] = F
    Fpad[n:] = np.array([1.0, 0.1, 0.0, 1.0], np.float32)

    in_maps = []
    for i in range(NCORES):
        m = {"F": np.ascontiguousarray(Fpad[i * NPC : (i + 1) * NPC])}
        m.update(wmaps)
        in_maps.append(m)

    res = run_bass_kernel_spmd(nc, in_maps, core_ids=list(range(NCORES)))
    _last_results = res
    out = np.concatenate([r["OUT"] for r in res.results], axis=0)[:n]
    return out.reshape(n, 2, 2).astype(np.float32)

